# revision 25
# baseline (speedup 1.0000x reference)
"""Trainium2 Bass kernel for nn_FCGF_point_att_k (ragged segment attention pooling).

Math (per segment b of n=16384 points, full N=262144, C=32, F1=256, F2=1024):
    h   = relu(bn1(x @ w1.T + b1))                 # [n, 256]
    att = relu(bn2(h @ w2.T + b2))                 # [n, 1024]
    soft = softmax_over_points(att)                # per channel k
    res[b] = sum_p mean_k(soft[p,k]) * x[p] / n    -> l2-normalize rows

Key reductions used by this kernel:
  * BN folds into the GEMM weights/bias on the host (eval mode).
  * softmax max-subtraction is unnecessary for this value range; with
    e[p,k] = max(exp(z[p,k]), exp(-c2[k]))  (z = h @ W2fold, c2 = folded
    bias), e equals exp(max(z,-c2)) = exp(-c2[k])*exp(relu(z+c2)) whose
    per-channel scale cancels in M[c,k]/den[k].
  * the whole output reduces to G = [x | 1].T @ e per segment ([33, 1024]):
    rows 0..31 = M, row 32 = den; res[c] = sum_k M[c,k]/den[k] (host, tiny).

Pipeline structure: the kernel is ScalarE-bound — exp is the only op
that must touch every [N, 1024] element on the one engine that has it, at
1 elem/cycle/lane with ~310 cycles of per-instruction overhead.  So the
exp granularity is widened from 1024 to 1536 columns (1.5 chunks of 128
points) by carving PSUM as: att [128,1536]x2 (6 banks) + l1 [128,512]x1
(1 bank) + gt (1 bank) = all 8 banks.  The half-chunk stream (chunk c,
channel-half kk) maps to att/e tiles of 3 x 512 columns.  Steady state is
ScalarE ~1589ns per tile (ACTIVATE 1538 + ~50ns semaphore handoff), with
PE ~90% busy (fp8 DoubleRow L2 at rate + G pairs + L1) and DVE ~80%.

PE-queue ordering is tuned for that handoff: per tile the three DR
matmuls are emitted first (they gate the next ACTIVATE), then the L1
prefetch, then the G pairs unlocked by the previous tile's clamp.  G
matmul pairs are emitted only when both kk halves of a chunk are
clamped, keeping the two M=33 matmuls adjacent so they run concurrently
in disjoint PE column groups (partitions 0..32 / 64..96).

Sharding: data-parallel, 2 whole segments per core on 8 cores; weights
replicated; per-core result is G [2, 33, 1024]; host combines.
"""

import numpy as np

# Problem shape (hardcoded per harness contract)
N, C_IN, F1, F2, B = 262144, 32, 256, 1024, 16
SEG = 16384
NCORES = 8
SEG_PER_CORE = B // NCORES          # 2
PTS = SEG_PER_CORE * SEG            # 32768 points per core
CH_PER_SEG = SEG // 128             # 128 chunks of 128 points per segment
NCHUNK = PTS // 128                 # 256 chunks per core
HC = NCHUNK * 2                     # 512 half-chunks (chunk, kk) of 512 cols
TW = 1536                           # ACT tile width (3 half-chunks, 3 PSUM banks)
NT = (HC + 2) // 3                  # 171 att/e tiles (last is 2 half-chunks)
BLK = 512                           # L1 block (points)
NBLK = PTS // BLK                   # 64
XT_TILE = 2048                      # streamed xT tile width (points)
EPS_BN = 1e-5
EPS_NORM = 1e-12
SH = 8.0                            # h pre-scale (folded into w1a)
SW = 16.0                           # w2 pre-scale; exp scale = 1/(SH*SW)

_NC_CACHE = {}


def _build():
    """Build + compile the per-core Bass program."""
    import concourse.bacc as bacc
    import concourse.mybir as mybir
    import concourse.tile as tile

    F32 = mybir.dt.float32
    BF16 = mybir.dt.bfloat16
    FP8 = mybir.dt.float8e4
    ACT = mybir.ActivationFunctionType
    DRSW = mybir.MatmulPerfMode.DoubleRowSwInterleave

    nc = bacc.Bacc("TRN2", target_bir_lowering=False, debug=False)
    # xa is pre-transposed on the host to partition-major [128, NCHUNK*33]
    # so the DMA is contiguous per partition.
    d_xa = nc.dram_tensor("xa", [128, NCHUNK, 33], BF16, kind="ExternalInput")
    d_xt = nc.dram_tensor("xt", [SEG_PER_CORE, 33, SEG], BF16, kind="ExternalInput")
    d_w1 = nc.dram_tensor("w1a", [33, 2, 128], BF16, kind="ExternalInput")
    d_w2 = nc.dram_tensor("w2f", [128, 2, F2], FP8, kind="ExternalInput")
    d_qa = nc.dram_tensor("qexpa", [128, TW], BF16, kind="ExternalInput")
    d_qb = nc.dram_tensor("qexpb", [128, TW], BF16, kind="ExternalInput")
    d_out = nc.dram_tensor("gout", [SEG_PER_CORE, 128, 512], F32, kind="ExternalOutput")

    with tile.TileContext(nc) as tc:
        with (
            tc.tile_pool(name="consts", bufs=1) as consts,
            tc.tile_pool(name="xtp", bufs=3) as xtp,
            tc.tile_pool(name="hp", bufs=3) as hp,
            tc.tile_pool(name="ep", bufs=4) as ep,
            tc.tile_pool(name="gop", bufs=2) as gop,
            tc.tile_pool(name="l1ps", bufs=1, space="PSUM") as l1ps,
            tc.tile_pool(name="attps", bufs=2, space="PSUM") as attps,
            tc.tile_pool(name="gps", bufs=1, space="PSUM") as gps,
        ):

            # DMA issue order follows per-queue FIFO, so order by deadline:
            # w1a + xt block 0 gate the first L1 matmul (block 0 is split
            # out of the xt0 transfer so L1(0) does not wait for the full
            # 2048-point tile); w2f gates the first L2; qa/qb gate the
            # first DVE clamp; xa pieces gate the G matmuls and go last.
            w1a = consts.tile([33, 2, 128], BF16)
            nc.sync.dma_start(out=w1a, in_=d_w1.ap())
            xt0 = xtp.tile([33, XT_TILE], BF16, tag="xt")
            nc.sync.dma_start(out=xt0[:, 0:BLK], in_=d_xt.ap()[0, :, 0:BLK])
            # w2f rides the sync queue between the xt0 pieces: on the gpsimd
            # queue its completion semaphore lagged to ~14.6us and became
            # the first DR matmul's gate (~2.3us of head).
            w2f = consts.tile([128, 2, F2], FP8)
            nc.sync.dma_start(out=w2f, in_=d_w2.ap())
            nc.sync.dma_start(out=xt0[:, BLK:XT_TILE], in_=d_xt.ap()[0, :, BLK:XT_TILE])
            qa = consts.tile([128, TW], BF16)
            nc.gpsimd.dma_start(out=qa, in_=d_qa.ap())
            qb = consts.tile([128, TW], BF16)
            nc.gpsimd.dma_start(out=qb, in_=d_qb.ap())
            xa_splits = [0, 8, 32, 64, 128, 192, NCHUNK]
            xa_tiles = []
            for sp in range(len(xa_splits) - 1):
                lo, hi = xa_splits[sp], xa_splits[sp + 1]
                t_ = consts.tile([128, hi - lo, 33], BF16, tag=f"xa{sp}")
                eng = nc.sync if sp % 2 == 0 else nc.gpsimd
                eng.dma_start(out=t_, in_=d_xa.ap()[:, lo:hi, :])
                xa_tiles.append(t_)

            def xa_chunk(gc):
                for sp in range(len(xa_splits) - 1):
                    if gc < xa_splits[sp + 1]:
                        return xa_tiles[sp][:, gc - xa_splits[sp], :]
                raise AssertionError

            xts = [xt0] + [None] * (PTS // XT_TILE - 1)
            hts = [None] * NBLK

            def emit_l1(b, f1c, pool=None, tag="l1"):
                # h for block b: l1 = w1a[f1c] @ xt-block -> relu -> fp8 SWI
                # ht[f, sub, 2*(127-m)+j] = relu(l1)[f1=128j+f, point m]
                if f1c == 0:
                    hts[b] = hp.tile([128, 4, 256], FP8, tag="ht", name="ht")
                l1t = (pool or l1ps).tile([128, BLK], F32, tag=tag, name="l1t")
                nc.tensor.matmul(
                    l1t,
                    w1a[:, f1c, :],
                    xts[b // 4][:, (b % 4) * BLK : (b % 4 + 1) * BLK],
                    start=True,
                    stop=True,
                )
                # relu + fp8 cast on DVE keeps ScalarE exp-only
                nc.vector.tensor_scalar_max(
                    hts[b][:, :, 254 + f1c :: -2], l1t, 0.0
                )

            # Block 0's two L1 tiles borrow idle att-pool banks so the
            # single-buffered l1 slot doesn't serialize the cold start:
            # both matmuls issue back-to-back into different banks.  Its
            # relus are also split per sub-chunk so ht sub 0 (all the first
            # DR needs) lands ~0.9us before the full 1024-column relu would.
            hts[0] = hp.tile([128, 4, 256], FP8, tag="ht", name="ht")
            l1b0 = []
            for f1c in range(2):
                l1t0 = attps.tile([128, BLK], F32, tag="att", name="l1t0")
                nc.tensor.matmul(
                    l1t0, w1a[:, f1c, :], xt0[:, 0:BLK], start=True, stop=True
                )
                l1b0.append(l1t0)
            for sub_lo, sub_hi in ((0, 1), (1, 4)):
                for f1c in range(2):
                    nc.vector.tensor_scalar_max(
                        hts[0][:, sub_lo:sub_hi, 254 + f1c :: -2],
                        l1b0[f1c][:, sub_lo * 128 : sub_hi * 128],
                        0.0,
                    )

            eq = {}          # half-chunk -> (e tile, col block)
            gt = gps.tile([128, 512], F32, tag="gt", name="gt")
            gq = []          # half-chunks whose G matmul is ready (clamped)
            l1q = [(1, 0), (1, 1)]   # L1 prefetch queue, <=1 served/section

            def flush_g():
                # G matmuls are emitted per half-chunk, 3 per section: the
                # PE's single PSUM write port (1 column/cycle) serializes
                # column-group pairs anyway, so pair adjacency buys nothing
                # and per-half-chunk emission keeps the PE column load
                # perfectly uniform across sections (no PE/ACT beat).
                nonlocal gt
                for hc in gq:
                    c, kk = divmod(hc, 2)
                    cl = c % CH_PER_SEG
                    et, col = eq.pop(hc)
                    nc.tensor.matmul(
                        gt[64 * kk : 64 * kk + 33, :],
                        xa_chunk(c),
                        et[:, col * 512 : (col + 1) * 512],
                        start=(cl == 0),
                        stop=(cl == CH_PER_SEG - 1),
                    )
                    if cl == CH_PER_SEG - 1 and kk == 1:
                        seg = c // CH_PER_SEG
                        gsb = gop.tile([128, 512], F32, tag="gsb", name="gsb")
                        nc.vector.tensor_copy(gsb, gt)
                        nc.sync.dma_start(out=d_out.ap()[seg], in_=gsb)
                        if seg + 1 < SEG_PER_CORE:
                            gt = gps.tile([128, 512], F32, tag="gt", name="gt")
                gq.clear()

            for t in range(NT):
                q0, q1 = 3 * t, min(3 * t + 3, HC)
                w = (q1 - q0) * 512
                att = attps.tile([128, TW], F32, tag="att", name="att")
                for q in range(q0, q1):
                    c, kk = divmod(q, 2)
                    # stream next xt tile well before its L1 consumers
                    if kk == 0 and c % 16 == 8 and c // 16 + 1 < len(xts):
                        g = c // 16 + 1
                        xts[g] = xtp.tile([33, XT_TILE], BF16, tag="xt", name="xt_t")
                        nc.sync.dma_start(
                            out=xts[g],
                            in_=d_xt.ap()[g // 8, :, (g % 8) * XT_TILE : (g % 8 + 1) * XT_TILE],
                        )
                    # L1 prefetch requests two blocks ahead; served at most
                    # one matmul per section to keep the PE load smooth.
                    if q % 8 == 2 and q // 8 + 2 < NBLK:
                        l1q.append((q // 8 + 2, 0))
                    elif q % 8 == 5 and q // 8 + 2 < NBLK:
                        l1q.append((q // 8 + 2, 1))
                    # L2 GEMM half-chunk: fp8 DoubleRow, K=256 in one pass
                    nc.tensor.matmul(
                        att[:, (q - q0) * 512 : (q - q0 + 1) * 512],
                        hts[c // 4][:, c % 4, :],
                        w2f[:, :, kk * 512 : (kk + 1) * 512],
                        start=True,
                        stop=True,
                        perf_mode=DRSW,
                    )
                if l1q:
                    emit_l1(*l1q.pop(0))
                # G matmuls unlocked by the previous tile's clamp go after
                # this tile's DRs (which gate the next ACTIVATE).
                flush_g()
                # exp over the whole 3-bank tile: one wide ACTIVATE amortizes
                # the ~310-cycle ScalarE per-instruction overhead.
                e = ep.tile([128, TW], BF16, tag="e", name="e")
                nc.scalar.activation(e[:, :w], att[:, :w], ACT.Exp, scale=1.0 / (SH * SW))
                qv = qa if t % 2 == 0 else qb
                for q in range(q0, q1):
                    eq[q] = (e, q - q0)
                nc.vector.tensor_max(e[:, :w], e[:, :w], qv[:, :w])
                gq.extend(range(q0, q1))
            while l1q:
                emit_l1(*l1q.pop(0))
            flush_g()



    nc.compile()
    return nc


def _get_nc():
    if "v3" not in _NC_CACHE:
        _NC_CACHE["v3"] = _build()
    return _NC_CACHE["v3"]


def _prep_inputs(x, w1, b1, g1, be1, m1, v1, w2, b2, g2, be2, m2, v2):
    """Fold BN into GEMM weights, build per-core device input maps.

    w1a carries SH*(W1|c1) so relu(l1) = SH*h fits fp8e4 well; w2f carries
    SW*W2.T so the DR matmul yields SH*SW*z; exp applies scale=1/(SH*SW).
    qexp = exp(-c2): e = max(exp(z), exp(-c2)) == exp(max(z, -c2)).
    qexpa/qexpb are the 1536-wide channel layouts for even/odd e tiles
    (kk pattern [0,1,0] / [1,0,1]).
    """
    import ml_dtypes

    f32 = np.float32
    bf16 = ml_dtypes.bfloat16
    fp8 = ml_dtypes.float8_e4m3
    x = np.asarray(x, f32)
    s1 = np.asarray(g1, f32) / np.sqrt(np.asarray(v1, f32) + EPS_BN)
    c1 = np.asarray(b1, f32) * s1 + np.asarray(be1, f32) - np.asarray(m1, f32) * s1
    s2 = np.asarray(g2, f32) / np.sqrt(np.asarray(v2, f32) + EPS_BN)
    c2 = np.asarray(b2, f32) * s2 + np.asarray(be2, f32) - np.asarray(m2, f32) * s2
    W1 = np.asarray(w1, f32) * s1[:, None]          # [256, 32]
    W2 = np.asarray(w2, f32) * s2[:, None]          # [1024, 256]

    w1a = np.empty((33, 2, 128), f32)
    w1a[:32] = W1.T.reshape(32, 2, 128)
    w1a[32] = c1.reshape(2, 128)
    w1a = (w1a * SH).astype(bf16)
    w2f = np.ascontiguousarray(
        (W2.T * SW).reshape(2, 128, F2).transpose(1, 0, 2)
    ).astype(fp8)
    q = np.exp(-c2)                                  # [1024]
    qla = np.concatenate([q[:512], q[512:], q[:512]])          # kk [0,1,0]
    qlb = np.concatenate([q[512:], q[:512], q[512:]])          # kk [1,0,1]
    qexpa = np.ascontiguousarray(np.broadcast_to(qla, (128, TW))).astype(bf16)
    qexpb = np.ascontiguousarray(np.broadcast_to(qlb, (128, TW))).astype(bf16)

    in_maps = []
    for i in range(NCORES):
        xs = x[i * PTS : (i + 1) * PTS]
        xa = np.empty((PTS, 33), f32)
        xa[:, :32] = xs
        xa[:, 32] = 1.0
        xt = np.ascontiguousarray(
            xa.reshape(SEG_PER_CORE, SEG, 33).transpose(0, 2, 1)
        ).astype(bf16)  # [2, 33, 16384]
        in_maps.append(
            {
                "xa": np.ascontiguousarray(
                    xa.reshape(NCHUNK, 128, 33).transpose(1, 0, 2)
                ).astype(bf16),
                "xt": xt,
                "w1a": w1a,
                "w2f": w2f,
                "qexpa": qexpa,
                "qexpb": qexpb,
            }
        )
    return in_maps


def _postprocess(results, length):
    f32 = np.float32
    Gp = np.stack([r["gout"] for r in results]).reshape(B, 128, 512)
    G = np.concatenate([Gp[:, 0:33, :], Gp[:, 64:97, :]], axis=2)  # [B, 33, 1024]
    M = G[:, :32, :]
    den = G[:, 32, :]
    res = (M / den[:, None, :]).sum(-1) / F2
    res = res / np.asarray(length, f32)[:, None]
    nrm = np.sqrt((res * res).sum(1, keepdims=True))
    return (res / np.maximum(nrm, EPS_NORM)).astype(f32)


def run_on_device(inputs, trace=False, **kwargs):
    """Run the device portion; returns BassKernelResults."""
    from concourse.bass_utils import run_bass_kernel_spmd

    if trace:
        try:
            import ntff_hook  # noqa: PLC0415  # available only in the dev dir

            ntff_hook.install()
        except ImportError:
            pass
    in_maps = _prep_inputs(
        inputs["x"], inputs["w1"], inputs["b1"], inputs["g1"], inputs["be1"],
        inputs["m1"], inputs["v1"], inputs["w2"], inputs["b2"], inputs["g2"],
        inputs["be2"], inputs["m2"], inputs["v2"],
    )
    nc = _get_nc()
    res = run_bass_kernel_spmd(
        nc, in_maps, core_ids=list(range(NCORES)), trace=trace, **kwargs
    )
    return res


def kernel(x, length, w1, b1, g1, be1, m1, v1, w2, b2, g2, be2, m2, v2):
    inputs = dict(
        x=x, length=length, w1=w1, b1=b1, g1=g1, be1=be1, m1=m1, v1=v1,
        w2=w2, b2=b2, g2=g2, be2=be2, m2=m2, v2=v2,
    )
    res = run_on_device(inputs, trace=False)
    return _postprocess(res.results, length)


# revision 26
# speedup vs baseline: 1.0085x; 1.0085x over previous
"""Trainium2 Bass kernel for nn_FCGF_point_att_k (ragged segment attention pooling).

Math (per segment b of n=16384 points, full N=262144, C=32, F1=256, F2=1024):
    h   = relu(bn1(x @ w1.T + b1))                 # [n, 256]
    att = relu(bn2(h @ w2.T + b2))                 # [n, 1024]
    soft = softmax_over_points(att)                # per channel k
    res[b] = sum_p mean_k(soft[p,k]) * x[p] / n    -> l2-normalize rows

Key reductions used by this kernel:
  * BN folds into the GEMM weights/bias on the host (eval mode).
  * softmax max-subtraction is unnecessary for this value range; with
    e[p,k] = max(exp(z[p,k]), exp(-c2[k]))  (z = h @ W2fold, c2 = folded
    bias), e equals exp(max(z,-c2)) = exp(-c2[k])*exp(relu(z+c2)) whose
    per-channel scale cancels in M[c,k]/den[k].
  * the whole output reduces to G = [x | 1].T @ e per segment ([33, 1024]):
    rows 0..31 = M, row 32 = den; res[c] = sum_k M[c,k]/den[k] (host, tiny).

Pipeline structure: the kernel is ScalarE-bound — exp is the only op
that must touch every [N, 1024] element on the one engine that has it, at
1 elem/cycle/lane with ~310 cycles of per-instruction overhead.  So the
exp granularity is widened from 1024 to 1536 columns (1.5 chunks of 128
points) by carving PSUM as: att [128,1536]x2 (6 banks) + l1 [128,512]x1
(1 bank) + gt (1 bank) = all 8 banks.  The half-chunk stream (chunk c,
channel-half kk) maps to att/e tiles of 3 x 512 columns.  Steady state is
ScalarE ~1589ns per tile (ACTIVATE 1538 + ~50ns semaphore handoff), with
PE ~90% busy (fp8 DoubleRow L2 at rate + G pairs + L1) and DVE ~80%.

PE-queue ordering is tuned for that handoff: per tile the three DR
matmuls are emitted first (they gate the next ACTIVATE), then the L1
prefetch, then the G pairs unlocked by the previous tile's clamp.  G
matmul pairs are emitted only when both kk halves of a chunk are
clamped, keeping the two M=33 matmuls adjacent so they run concurrently
in disjoint PE column groups (partitions 0..32 / 64..96).

Sharding: data-parallel, 2 whole segments per core on 8 cores; weights
replicated; per-core result is G [2, 33, 1024]; host combines.
"""

import numpy as np

# Problem shape (hardcoded per harness contract)
N, C_IN, F1, F2, B = 262144, 32, 256, 1024, 16
SEG = 16384
NCORES = 8
SEG_PER_CORE = B // NCORES          # 2
PTS = SEG_PER_CORE * SEG            # 32768 points per core
CH_PER_SEG = SEG // 128             # 128 chunks of 128 points per segment
NCHUNK = PTS // 128                 # 256 chunks per core
HC = NCHUNK * 2                     # 512 half-chunks (chunk, kk) of 512 cols
TW = 1536                           # ACT tile width (3 half-chunks, 3 PSUM banks)
NT = (HC + 2) // 3                  # 171 att/e tiles (last is 2 half-chunks)
BLK = 512                           # L1 block (points)
NBLK = PTS // BLK                   # 64
XT_TILE = 2048                      # streamed xT tile width (points)
EPS_BN = 1e-5
EPS_NORM = 1e-12
SH = 8.0                            # h pre-scale (folded into w1a)
SW = 16.0                           # w2 pre-scale; exp scale = 1/(SH*SW)

_NC_CACHE = {}


def _build():
    """Build + compile the per-core Bass program."""
    import concourse.bacc as bacc
    import concourse.mybir as mybir
    import concourse.tile as tile

    F32 = mybir.dt.float32
    BF16 = mybir.dt.bfloat16
    FP8 = mybir.dt.float8e4
    ACT = mybir.ActivationFunctionType
    DRSW = mybir.MatmulPerfMode.DoubleRowSwInterleave

    nc = bacc.Bacc("TRN2", target_bir_lowering=False, debug=False)
    # xa is pre-transposed on the host to partition-major [128, NCHUNK*33]
    # so the DMA is contiguous per partition.
    d_xa = nc.dram_tensor("xa", [128, NCHUNK, 33], BF16, kind="ExternalInput")
    d_xt = nc.dram_tensor("xt", [SEG_PER_CORE, 33, SEG], BF16, kind="ExternalInput")
    d_w1 = nc.dram_tensor("w1a", [33, 2, 128], BF16, kind="ExternalInput")
    d_w2 = nc.dram_tensor("w2f", [128, 2, F2], FP8, kind="ExternalInput")
    d_qa = nc.dram_tensor("qexpa", [128, TW], BF16, kind="ExternalInput")
    d_qb = nc.dram_tensor("qexpb", [128, TW], BF16, kind="ExternalInput")
    d_out = nc.dram_tensor("gout", [SEG_PER_CORE, 128, 512], F32, kind="ExternalOutput")

    with tile.TileContext(nc) as tc:
        with (
            tc.tile_pool(name="consts", bufs=1) as consts,
            tc.tile_pool(name="xtp", bufs=3) as xtp,
            tc.tile_pool(name="hp", bufs=3) as hp,
            tc.tile_pool(name="ep", bufs=4) as ep,
            tc.tile_pool(name="gop", bufs=2) as gop,
            tc.tile_pool(name="l1ps", bufs=1, space="PSUM") as l1ps,
            tc.tile_pool(name="attps", bufs=2, space="PSUM") as attps,
            tc.tile_pool(name="gps", bufs=1, space="PSUM") as gps,
        ):

            # DMA issue order follows per-queue FIFO, so order by deadline:
            # w1a + xt block 0 gate the first L1 matmul (block 0 is split
            # out of the xt0 transfer so L1(0) does not wait for the full
            # 2048-point tile); w2f gates the first L2; qa/qb gate the
            # first DVE clamp; xa pieces gate the G matmuls and go last.
            w1a = consts.tile([33, 2, 128], BF16)
            nc.sync.dma_start(out=w1a, in_=d_w1.ap())
            xt0 = xtp.tile([33, XT_TILE], BF16, tag="xt")
            nc.sync.dma_start(out=xt0[:, 0:BLK], in_=d_xt.ap()[0, :, 0:BLK])
            # w2f rides the sync queue between the xt0 pieces: on the gpsimd
            # queue its completion semaphore lagged to ~14.6us and became
            # the first DR matmul's gate (~2.3us of head).
            w2f = consts.tile([128, 2, F2], FP8)
            nc.sync.dma_start(out=w2f, in_=d_w2.ap())
            nc.sync.dma_start(out=xt0[:, BLK:XT_TILE], in_=d_xt.ap()[0, :, BLK:XT_TILE])
            qa = consts.tile([128, TW], BF16)
            nc.gpsimd.dma_start(out=qa, in_=d_qa.ap())
            qb = consts.tile([128, TW], BF16)
            nc.gpsimd.dma_start(out=qb, in_=d_qb.ap())
            xa_splits = [0, 8, 32, 64, 128, 192, NCHUNK]
            xa_tiles = []
            for sp in range(len(xa_splits) - 1):
                lo, hi = xa_splits[sp], xa_splits[sp + 1]
                t_ = consts.tile([128, hi - lo, 33], BF16, tag=f"xa{sp}")
                eng = nc.sync if sp % 2 == 0 else nc.gpsimd
                eng.dma_start(out=t_, in_=d_xa.ap()[:, lo:hi, :])
                xa_tiles.append(t_)

            def xa_chunk(gc):
                for sp in range(len(xa_splits) - 1):
                    if gc < xa_splits[sp + 1]:
                        return xa_tiles[sp][:, gc - xa_splits[sp], :]
                raise AssertionError

            xts = [xt0] + [None] * (PTS // XT_TILE - 1)
            hts = [None] * NBLK

            def emit_l1(b, f1c, pool=None, tag="l1"):
                # h for block b: l1 = w1a[f1c] @ xt-block -> relu -> fp8 SWI
                # ht[f, sub, 2*(127-m)+j] = relu(l1)[f1=128j+f, point m]
                if f1c == 0:
                    hts[b] = hp.tile([128, 4, 256], FP8, tag="ht", name="ht")
                l1t = (pool or l1ps).tile([128, BLK], F32, tag=tag, name="l1t")
                nc.tensor.matmul(
                    l1t,
                    w1a[:, f1c, :],
                    xts[b // 4][:, (b % 4) * BLK : (b % 4 + 1) * BLK],
                    start=True,
                    stop=True,
                )
                # relu + fp8 cast on DVE keeps ScalarE exp-only
                nc.vector.tensor_scalar_max(
                    hts[b][:, :, 254 + f1c :: -2], l1t, 0.0
                )

            # Block 0's two L1 tiles borrow idle att-pool banks so the
            # single-buffered l1 slot doesn't serialize the cold start:
            # both matmuls issue back-to-back into different banks.  Its
            # relus are also split per sub-chunk so ht sub 0 (all the first
            # DR needs) lands ~0.9us before the full 1024-column relu would.
            hts[0] = hp.tile([128, 4, 256], FP8, tag="ht", name="ht")
            l1b0 = []
            for f1c in range(2):
                l1t0 = attps.tile([128, BLK], F32, tag="att", name="l1t0")
                nc.tensor.matmul(
                    l1t0, w1a[:, f1c, :], xt0[:, 0:BLK], start=True, stop=True
                )
                l1b0.append(l1t0)
            for sub_lo, sub_hi in ((0, 1), (1, 4)):
                for f1c in range(2):
                    nc.vector.tensor_scalar_max(
                        hts[0][:, sub_lo:sub_hi, 254 + f1c :: -2],
                        l1b0[f1c][:, sub_lo * 128 : sub_hi * 128],
                        0.0,
                    )

            eq = {}          # half-chunk -> (e tile, col block)
            gt = gps.tile([128, 512], F32, tag="gt", name="gt")
            next_g = 0       # next chunk whose G pair is pending
            pend = []        # G pairs ready to emit (deferred past next DRs)

            def flush_pairs():
                # G pair emission: the PE single PSUM write port serializes
                # the two M=33 matmuls regardless of column groups, so pair
                # shape is about bookkeeping, not overlap.  Pairs unlocked
                # by the previous tile's clamp go after this tile's DRs
                # (which gate the next ACTIVATE).
                nonlocal gt
                for c in pend:
                    cl = c % CH_PER_SEG
                    for kk in range(2):
                        et, col = eq.pop(2 * c + kk)
                        nc.tensor.matmul(
                            gt[64 * kk : 64 * kk + 33, :],
                            xa_chunk(c),
                            et[:, col * 512 : (col + 1) * 512],
                            start=(cl == 0),
                            stop=(cl == CH_PER_SEG - 1),
                        )
                    if cl == CH_PER_SEG - 1:
                        seg = c // CH_PER_SEG
                        gsb = gop.tile([128, 512], F32, tag="gsb", name="gsb")
                        nc.vector.tensor_copy(gsb, gt)
                        nc.sync.dma_start(out=d_out.ap()[seg], in_=gsb)
                        if seg + 1 < SEG_PER_CORE:
                            gt = gps.tile([128, 512], F32, tag="gt", name="gt")
                pend.clear()

            for t in range(NT):
                q0, q1 = 3 * t, min(3 * t + 3, HC)
                w = (q1 - q0) * 512
                att = attps.tile([128, TW], F32, tag="att", name="att")
                l1_pend = []
                for q in range(q0, q1):
                    c, kk = divmod(q, 2)
                    # stream next xt tile well before its L1 consumers
                    if kk == 0 and c % 16 == 8 and c // 16 + 1 < len(xts):
                        g = c // 16 + 1
                        xts[g] = xtp.tile([33, XT_TILE], BF16, tag="xt", name="xt_t")
                        nc.sync.dma_start(
                            out=xts[g],
                            in_=d_xt.ap()[g // 8, :, (g % 8) * XT_TILE : (g % 8 + 1) * XT_TILE],
                        )
                    # prefetch L1 for block b+1, split across the block
                    # window; deferred below the DRs so the DR gating the
                    # next ACTIVATE is never stuck behind an L1 matmul.
                    if q % 8 == 2 and q // 8 + 1 < NBLK:
                        l1_pend.append((q // 8 + 1, 0))
                    elif q % 8 == 5 and q // 8 + 1 < NBLK:
                        l1_pend.append((q // 8 + 1, 1))
                    # L2 GEMM half-chunk: fp8 DoubleRow, K=256 in one pass
                    nc.tensor.matmul(
                        att[:, (q - q0) * 512 : (q - q0 + 1) * 512],
                        hts[c // 4][:, c % 4, :],
                        w2f[:, :, kk * 512 : (kk + 1) * 512],
                        start=True,
                        stop=True,
                        perf_mode=DRSW,
                    )
                for b_, f_ in l1_pend:
                    emit_l1(b_, f_)
                flush_pairs()
                # exp over the whole 3-bank tile: one wide ACTIVATE amortizes
                # the ~310-cycle ScalarE per-instruction overhead.
                e = ep.tile([128, TW], BF16, tag="e", name="e")
                nc.scalar.activation(e[:, :w], att[:, :w], ACT.Exp, scale=1.0 / (SH * SW))
                qv = qa if t % 2 == 0 else qb
                for q in range(q0, q1):
                    eq[q] = (e, q - q0)
                nc.vector.tensor_max(e[:, :w], e[:, :w], qv[:, :w])
                while next_g * 2 + 1 <= q1 - 1:
                    pend.append(next_g)
                    next_g += 1
            flush_pairs()



    nc.compile()
    return nc


def _get_nc():
    if "v3" not in _NC_CACHE:
        _NC_CACHE["v3"] = _build()
    return _NC_CACHE["v3"]


def _prep_inputs(x, w1, b1, g1, be1, m1, v1, w2, b2, g2, be2, m2, v2):
    """Fold BN into GEMM weights, build per-core device input maps.

    w1a carries SH*(W1|c1) so relu(l1) = SH*h fits fp8e4 well; w2f carries
    SW*W2.T so the DR matmul yields SH*SW*z; exp applies scale=1/(SH*SW).
    qexp = exp(-c2): e = max(exp(z), exp(-c2)) == exp(max(z, -c2)).
    qexpa/qexpb are the 1536-wide channel layouts for even/odd e tiles
    (kk pattern [0,1,0] / [1,0,1]).
    """
    import ml_dtypes

    f32 = np.float32
    bf16 = ml_dtypes.bfloat16
    fp8 = ml_dtypes.float8_e4m3
    x = np.asarray(x, f32)
    s1 = np.asarray(g1, f32) / np.sqrt(np.asarray(v1, f32) + EPS_BN)
    c1 = np.asarray(b1, f32) * s1 + np.asarray(be1, f32) - np.asarray(m1, f32) * s1
    s2 = np.asarray(g2, f32) / np.sqrt(np.asarray(v2, f32) + EPS_BN)
    c2 = np.asarray(b2, f32) * s2 + np.asarray(be2, f32) - np.asarray(m2, f32) * s2
    W1 = np.asarray(w1, f32) * s1[:, None]          # [256, 32]
    W2 = np.asarray(w2, f32) * s2[:, None]          # [1024, 256]

    w1a = np.empty((33, 2, 128), f32)
    w1a[:32] = W1.T.reshape(32, 2, 128)
    w1a[32] = c1.reshape(2, 128)
    w1a = (w1a * SH).astype(bf16)
    w2f = np.ascontiguousarray(
        (W2.T * SW).reshape(2, 128, F2).transpose(1, 0, 2)
    ).astype(fp8)
    q = np.exp(-c2)                                  # [1024]
    qla = np.concatenate([q[:512], q[512:], q[:512]])          # kk [0,1,0]
    qlb = np.concatenate([q[512:], q[:512], q[512:]])          # kk [1,0,1]
    qexpa = np.ascontiguousarray(np.broadcast_to(qla, (128, TW))).astype(bf16)
    qexpb = np.ascontiguousarray(np.broadcast_to(qlb, (128, TW))).astype(bf16)

    in_maps = []
    for i in range(NCORES):
        xs = x[i * PTS : (i + 1) * PTS]
        xa = np.empty((PTS, 33), f32)
        xa[:, :32] = xs
        xa[:, 32] = 1.0
        xt = np.ascontiguousarray(
            xa.reshape(SEG_PER_CORE, SEG, 33).transpose(0, 2, 1)
        ).astype(bf16)  # [2, 33, 16384]
        in_maps.append(
            {
                "xa": np.ascontiguousarray(
                    xa.reshape(NCHUNK, 128, 33).transpose(1, 0, 2)
                ).astype(bf16),
                "xt": xt,
                "w1a": w1a,
                "w2f": w2f,
                "qexpa": qexpa,
                "qexpb": qexpb,
            }
        )
    return in_maps


def _postprocess(results, length):
    f32 = np.float32
    Gp = np.stack([r["gout"] for r in results]).reshape(B, 128, 512)
    G = np.concatenate([Gp[:, 0:33, :], Gp[:, 64:97, :]], axis=2)  # [B, 33, 1024]
    M = G[:, :32, :]
    den = G[:, 32, :]
    res = (M / den[:, None, :]).sum(-1) / F2
    res = res / np.asarray(length, f32)[:, None]
    nrm = np.sqrt((res * res).sum(1, keepdims=True))
    return (res / np.maximum(nrm, EPS_NORM)).astype(f32)


def run_on_device(inputs, trace=False, **kwargs):
    """Run the device portion; returns BassKernelResults."""
    from concourse.bass_utils import run_bass_kernel_spmd

    if trace:
        try:
            import ntff_hook  # noqa: PLC0415  # available only in the dev dir

            ntff_hook.install()
        except ImportError:
            pass
    in_maps = _prep_inputs(
        inputs["x"], inputs["w1"], inputs["b1"], inputs["g1"], inputs["be1"],
        inputs["m1"], inputs["v1"], inputs["w2"], inputs["b2"], inputs["g2"],
        inputs["be2"], inputs["m2"], inputs["v2"],
    )
    nc = _get_nc()
    res = run_bass_kernel_spmd(
        nc, in_maps, core_ids=list(range(NCORES)), trace=trace, **kwargs
    )
    return res


def kernel(x, length, w1, b1, g1, be1, m1, v1, w2, b2, g2, be2, m2, v2):
    inputs = dict(
        x=x, length=length, w1=w1, b1=b1, g1=g1, be1=be1, m1=m1, v1=v1,
        w2=w2, b2=b2, g2=g2, be2=be2, m2=m2, v2=v2,
    )
    res = run_on_device(inputs, trace=False)
    return _postprocess(res.results, length)


# revision 27
# speedup vs baseline: 1.0096x; 1.0011x over previous
"""Trainium2 Bass kernel for nn_FCGF_point_att_k (ragged segment attention pooling).

Math (per segment b of n=16384 points, full N=262144, C=32, F1=256, F2=1024):
    h   = relu(bn1(x @ w1.T + b1))                 # [n, 256]
    att = relu(bn2(h @ w2.T + b2))                 # [n, 1024]
    soft = softmax_over_points(att)                # per channel k
    res[b] = sum_p mean_k(soft[p,k]) * x[p] / n    -> l2-normalize rows

Key reductions used by this kernel:
  * BN folds into the GEMM weights/bias on the host (eval mode).
  * softmax max-subtraction is unnecessary for this value range; with
    e[p,k] = max(exp(z[p,k]), exp(-c2[k]))  (z = h @ W2fold, c2 = folded
    bias), e equals exp(max(z,-c2)) = exp(-c2[k])*exp(relu(z+c2)) whose
    per-channel scale cancels in M[c,k]/den[k].
  * the whole output reduces to G = [x | 1].T @ e per segment ([33, 1024]):
    rows 0..31 = M, row 32 = den; res[c] = sum_k M[c,k]/den[k] (host, tiny).

Pipeline structure: the kernel saturates TWO engines at the same rate.
ScalarE: exp is the only op that must touch every [N, 1024] element on
the one engine that has it, at 1 elem/cycle/lane; widening the exp
granularity to 1536 columns (1.5 chunks of 128 points) amortizes its
~310-cycle per-instruction overhead.  PSUM is carved as att [128,1536]x2
(6 banks) + l1 [128,512]x1 (1 bank) + gt (1 bank) = all 8 banks, which
caps the ACTIVATE width.  PE: its single PSUM write port drains one
output column per cycle, and each 3-half-chunk section produces 3x512
(L2-DR) + 3x512 (G) + 0.75x512 (L1) = ~3456 columns = ~1440ns @2.4GHz
plus ~150ns of issue bubbles — the same ~1590ns/section as ScalarE's
ACTIVATE(1536)+handoff.  Both at rate; neither dominates.

Per tile the three DR matmuls are emitted first (they gate the next
ACTIVATE), then the L1 prefetch, then the G pairs unlocked by the
previous tile's clamp.  Note the two M=33 G matmuls of a pair do NOT
speed up via disjoint PE column groups — the single PSUM write port
serializes their output columns regardless; pair emission is just
bookkeeping.  G matmuls cannot merge across chunks (disjoint 128-point
contraction sets; K<=128), so the G column count is formulation-minimal.

Sharding: data-parallel, 2 whole segments per core on 8 cores; weights
replicated; per-core result is G [2, 33, 1024]; host combines.
"""

import numpy as np

# Problem shape (hardcoded per harness contract)
N, C_IN, F1, F2, B = 262144, 32, 256, 1024, 16
SEG = 16384
NCORES = 8
SEG_PER_CORE = B // NCORES          # 2
PTS = SEG_PER_CORE * SEG            # 32768 points per core
CH_PER_SEG = SEG // 128             # 128 chunks of 128 points per segment
NCHUNK = PTS // 128                 # 256 chunks per core
HC = NCHUNK * 2                     # 512 half-chunks (chunk, kk) of 512 cols
TW = 1536                           # ACT tile width (3 half-chunks, 3 PSUM banks)
NT = (HC + 2) // 3                  # 171 att/e tiles (last is 2 half-chunks)
BLK = 512                           # L1 block (points)
NBLK = PTS // BLK                   # 64
XT_TILE = 2048                      # streamed xT tile width (points)
EPS_BN = 1e-5
EPS_NORM = 1e-12
SH = 8.0                            # h pre-scale (folded into w1a)
SW = 16.0                           # w2 pre-scale; exp scale = 1/(SH*SW)

_NC_CACHE = {}


def _build():
    """Build + compile the per-core Bass program."""
    import concourse.bacc as bacc
    import concourse.mybir as mybir
    import concourse.tile as tile

    F32 = mybir.dt.float32
    BF16 = mybir.dt.bfloat16
    FP8 = mybir.dt.float8e4
    ACT = mybir.ActivationFunctionType
    DRSW = mybir.MatmulPerfMode.DoubleRowSwInterleave

    nc = bacc.Bacc("TRN2", target_bir_lowering=False, debug=False)
    # xa is pre-transposed on the host to partition-major [128, NCHUNK*33]
    # so the DMA is contiguous per partition.
    d_xa = nc.dram_tensor("xa", [128, NCHUNK, 33], BF16, kind="ExternalInput")
    d_xt = nc.dram_tensor("xt", [SEG_PER_CORE, 33, SEG], BF16, kind="ExternalInput")
    d_w1 = nc.dram_tensor("w1a", [33, 2, 128], BF16, kind="ExternalInput")
    d_w2 = nc.dram_tensor("w2f", [128, 2, F2], FP8, kind="ExternalInput")
    d_qa = nc.dram_tensor("qexpa", [128, TW], BF16, kind="ExternalInput")
    d_qb = nc.dram_tensor("qexpb", [128, TW], BF16, kind="ExternalInput")
    d_out = nc.dram_tensor("gout", [SEG_PER_CORE, 128, 512], F32, kind="ExternalOutput")

    with tile.TileContext(nc) as tc:
        with (
            tc.tile_pool(name="consts", bufs=1) as consts,
            tc.tile_pool(name="xtp", bufs=3) as xtp,
            tc.tile_pool(name="hp", bufs=3) as hp,
            tc.tile_pool(name="ep", bufs=4) as ep,
            tc.tile_pool(name="gop", bufs=2) as gop,
            tc.tile_pool(name="l1ps", bufs=1, space="PSUM") as l1ps,
            tc.tile_pool(name="attps", bufs=2, space="PSUM") as attps,
            tc.tile_pool(name="gps", bufs=1, space="PSUM") as gps,
        ):

            # DMA issue order follows per-queue FIFO, so order by deadline:
            # w1a + xt block 0 gate the first L1 matmul (block 0 is split
            # out of the xt0 transfer so L1(0) does not wait for the full
            # 2048-point tile); w2f gates the first L2; qa/qb gate the
            # first DVE clamp; xa pieces gate the G matmuls and go last.
            w1a = consts.tile([33, 2, 128], BF16)
            nc.sync.dma_start(out=w1a, in_=d_w1.ap())
            xt0 = xtp.tile([33, XT_TILE], BF16, tag="xt")
            nc.sync.dma_start(out=xt0[:, 0:BLK], in_=d_xt.ap()[0, :, 0:BLK])
            # w2f rides the sync queue between the xt0 pieces: on the gpsimd
            # queue its completion semaphore lagged to ~14.6us and became
            # the first DR matmul's gate (~2.3us of head).
            w2f = consts.tile([128, 2, F2], FP8)
            nc.sync.dma_start(out=w2f, in_=d_w2.ap())
            nc.sync.dma_start(out=xt0[:, BLK:XT_TILE], in_=d_xt.ap()[0, :, BLK:XT_TILE])
            qa = consts.tile([128, TW], BF16)
            nc.gpsimd.dma_start(out=qa, in_=d_qa.ap())
            qb = consts.tile([128, TW], BF16)
            nc.gpsimd.dma_start(out=qb, in_=d_qb.ap())
            xa_splits = [0, 8, 32, 64, 128, 192, NCHUNK]
            xa_tiles = []
            for sp in range(len(xa_splits) - 1):
                lo, hi = xa_splits[sp], xa_splits[sp + 1]
                t_ = consts.tile([128, hi - lo, 33], BF16, tag=f"xa{sp}")
                eng = nc.sync if sp % 2 == 0 else nc.gpsimd
                eng.dma_start(out=t_, in_=d_xa.ap()[:, lo:hi, :])
                xa_tiles.append(t_)

            def xa_chunk(gc):
                for sp in range(len(xa_splits) - 1):
                    if gc < xa_splits[sp + 1]:
                        return xa_tiles[sp][:, gc - xa_splits[sp], :]
                raise AssertionError

            xts = [xt0] + [None] * (PTS // XT_TILE - 1)
            hts = [None] * NBLK

            def emit_l1(b, f1c, pool=None, tag="l1"):
                # h for block b: l1 = w1a[f1c] @ xt-block -> relu -> fp8 SWI
                # ht[f, sub, 2*(127-m)+j] = relu(l1)[f1=128j+f, point m]
                if f1c == 0:
                    hts[b] = hp.tile([128, 4, 256], FP8, tag="ht", name="ht")
                l1t = (pool or l1ps).tile([128, BLK], F32, tag=tag, name="l1t")
                nc.tensor.matmul(
                    l1t,
                    w1a[:, f1c, :],
                    xts[b // 4][:, (b % 4) * BLK : (b % 4 + 1) * BLK],
                    start=True,
                    stop=True,
                )
                # relu + fp8 cast on DVE keeps ScalarE exp-only
                nc.vector.tensor_scalar_max(
                    hts[b][:, :, 254 + f1c :: -2], l1t, 0.0
                )

            # Block 0's two L1 tiles borrow idle att-pool banks so the
            # single-buffered l1 slot doesn't serialize the cold start:
            # both matmuls issue back-to-back into different banks.  Its
            # relus are also split per sub-chunk so ht sub 0 (all the first
            # DR needs) lands ~0.9us before the full 1024-column relu would.
            hts[0] = hp.tile([128, 4, 256], FP8, tag="ht", name="ht")
            l1b0 = []
            for f1c in range(2):
                l1t0 = attps.tile([128, BLK], F32, tag="att", name="l1t0")
                nc.tensor.matmul(
                    l1t0, w1a[:, f1c, :], xt0[:, 0:BLK], start=True, stop=True
                )
                l1b0.append(l1t0)
            for sub_lo, sub_hi in ((0, 1), (1, 4)):
                for f1c in range(2):
                    nc.vector.tensor_scalar_max(
                        hts[0][:, sub_lo:sub_hi, 254 + f1c :: -2],
                        l1b0[f1c][:, sub_lo * 128 : sub_hi * 128],
                        0.0,
                    )

            eq = {}          # half-chunk -> (e tile, col block)
            gt = gps.tile([128, 512], F32, tag="gt", name="gt")
            next_g = 0       # next chunk whose G pair is pending
            pend = []        # G pairs ready to emit (deferred past next DRs)

            def flush_pairs():
                # G pair emission: the PE single PSUM write port serializes
                # the two M=33 matmuls regardless of column groups, so pair
                # shape is about bookkeeping, not overlap.  Pairs unlocked
                # by the previous tile's clamp go after this tile's DRs
                # (which gate the next ACTIVATE).
                nonlocal gt
                for c in pend:
                    cl = c % CH_PER_SEG
                    for kk in range(2):
                        et, col = eq.pop(2 * c + kk)
                        nc.tensor.matmul(
                            gt[64 * kk : 64 * kk + 33, :],
                            xa_chunk(c),
                            et[:, col * 512 : (col + 1) * 512],
                            start=(cl == 0),
                            stop=(cl == CH_PER_SEG - 1),
                        )
                    if cl == CH_PER_SEG - 1:
                        seg = c // CH_PER_SEG
                        gsb = gop.tile([128, 512], F32, tag="gsb", name="gsb")
                        nc.vector.tensor_copy(gsb, gt)
                        nc.sync.dma_start(out=d_out.ap()[seg], in_=gsb)
                        if seg + 1 < SEG_PER_CORE:
                            gt = gps.tile([128, 512], F32, tag="gt", name="gt")
                pend.clear()

            for t in range(NT):
                q0, q1 = 3 * t, min(3 * t + 3, HC)
                w = (q1 - q0) * 512
                att = attps.tile([128, TW], F32, tag="att", name="att")
                l1_pend = []
                for q in range(q0, q1):
                    c, kk = divmod(q, 2)
                    # stream next xt tile well before its L1 consumers
                    if kk == 0 and c % 16 == 8 and c // 16 + 1 < len(xts):
                        g = c // 16 + 1
                        xts[g] = xtp.tile([33, XT_TILE], BF16, tag="xt", name="xt_t")
                        nc.sync.dma_start(
                            out=xts[g],
                            in_=d_xt.ap()[g // 8, :, (g % 8) * XT_TILE : (g % 8 + 1) * XT_TILE],
                        )
                    # prefetch L1 for block b+1, split across the block
                    # window; deferred below the DRs so the DR gating the
                    # next ACTIVATE is never stuck behind an L1 matmul.
                    if q % 8 == 2 and q // 8 + 1 < NBLK:
                        l1_pend.append((q // 8 + 1, 0))
                    elif q % 8 == 5 and q // 8 + 1 < NBLK:
                        l1_pend.append((q // 8 + 1, 1))
                    # L2 GEMM half-chunk: fp8 DoubleRow, K=256 in one pass
                    nc.tensor.matmul(
                        att[:, (q - q0) * 512 : (q - q0 + 1) * 512],
                        hts[c // 4][:, c % 4, :],
                        w2f[:, :, kk * 512 : (kk + 1) * 512],
                        start=True,
                        stop=True,
                        perf_mode=DRSW,
                    )
                for b_, f_ in l1_pend:
                    emit_l1(b_, f_)
                flush_pairs()
                # exp over the whole 3-bank tile: one wide ACTIVATE amortizes
                # the ~310-cycle ScalarE per-instruction overhead.
                e = ep.tile([128, TW], BF16, tag="e", name="e")
                nc.scalar.activation(e[:, :w], att[:, :w], ACT.Exp, scale=1.0 / (SH * SW))
                qv = qa if t % 2 == 0 else qb
                for q in range(q0, q1):
                    eq[q] = (e, q - q0)
                nc.vector.tensor_max(e[:, :w], e[:, :w], qv[:, :w])
                while next_g * 2 + 1 <= q1 - 1:
                    pend.append(next_g)
                    next_g += 1
            flush_pairs()



    nc.compile()
    return nc


def _get_nc():
    if "v3" not in _NC_CACHE:
        _NC_CACHE["v3"] = _build()
    return _NC_CACHE["v3"]


def _prep_inputs(x, w1, b1, g1, be1, m1, v1, w2, b2, g2, be2, m2, v2):
    """Fold BN into GEMM weights, build per-core device input maps.

    w1a carries SH*(W1|c1) so relu(l1) = SH*h fits fp8e4 well; w2f carries
    SW*W2.T so the DR matmul yields SH*SW*z; exp applies scale=1/(SH*SW).
    qexp = exp(-c2): e = max(exp(z), exp(-c2)) == exp(max(z, -c2)).
    qexpa/qexpb are the 1536-wide channel layouts for even/odd e tiles
    (kk pattern [0,1,0] / [1,0,1]).
    """
    import ml_dtypes

    f32 = np.float32
    bf16 = ml_dtypes.bfloat16
    fp8 = ml_dtypes.float8_e4m3
    x = np.asarray(x, f32)
    s1 = np.asarray(g1, f32) / np.sqrt(np.asarray(v1, f32) + EPS_BN)
    c1 = np.asarray(b1, f32) * s1 + np.asarray(be1, f32) - np.asarray(m1, f32) * s1
    s2 = np.asarray(g2, f32) / np.sqrt(np.asarray(v2, f32) + EPS_BN)
    c2 = np.asarray(b2, f32) * s2 + np.asarray(be2, f32) - np.asarray(m2, f32) * s2
    W1 = np.asarray(w1, f32) * s1[:, None]          # [256, 32]
    W2 = np.asarray(w2, f32) * s2[:, None]          # [1024, 256]

    w1a = np.empty((33, 2, 128), f32)
    w1a[:32] = W1.T.reshape(32, 2, 128)
    w1a[32] = c1.reshape(2, 128)
    w1a = (w1a * SH).astype(bf16)
    w2f = np.ascontiguousarray(
        (W2.T * SW).reshape(2, 128, F2).transpose(1, 0, 2)
    ).astype(fp8)
    q = np.exp(-c2)                                  # [1024]
    qla = np.concatenate([q[:512], q[512:], q[:512]])          # kk [0,1,0]
    qlb = np.concatenate([q[512:], q[:512], q[512:]])          # kk [1,0,1]
    qexpa = np.ascontiguousarray(np.broadcast_to(qla, (128, TW))).astype(bf16)
    qexpb = np.ascontiguousarray(np.broadcast_to(qlb, (128, TW))).astype(bf16)

    in_maps = []
    for i in range(NCORES):
        xs = x[i * PTS : (i + 1) * PTS]
        xa = np.empty((PTS, 33), f32)
        xa[:, :32] = xs
        xa[:, 32] = 1.0
        xt = np.ascontiguousarray(
            xa.reshape(SEG_PER_CORE, SEG, 33).transpose(0, 2, 1)
        ).astype(bf16)  # [2, 33, 16384]
        in_maps.append(
            {
                "xa": np.ascontiguousarray(
                    xa.reshape(NCHUNK, 128, 33).transpose(1, 0, 2)
                ).astype(bf16),
                "xt": xt,
                "w1a": w1a,
                "w2f": w2f,
                "qexpa": qexpa,
                "qexpb": qexpb,
            }
        )
    return in_maps


def _postprocess(results, length):
    f32 = np.float32
    Gp = np.stack([r["gout"] for r in results]).reshape(B, 128, 512)
    G = np.concatenate([Gp[:, 0:33, :], Gp[:, 64:97, :]], axis=2)  # [B, 33, 1024]
    M = G[:, :32, :]
    den = G[:, 32, :]
    res = (M / den[:, None, :]).sum(-1) / F2
    res = res / np.asarray(length, f32)[:, None]
    nrm = np.sqrt((res * res).sum(1, keepdims=True))
    return (res / np.maximum(nrm, EPS_NORM)).astype(f32)


def run_on_device(inputs, trace=False, **kwargs):
    """Run the device portion; returns BassKernelResults."""
    from concourse.bass_utils import run_bass_kernel_spmd

    if trace:
        try:
            import ntff_hook  # noqa: PLC0415  # available only in the dev dir

            ntff_hook.install()
        except ImportError:
            pass
    in_maps = _prep_inputs(
        inputs["x"], inputs["w1"], inputs["b1"], inputs["g1"], inputs["be1"],
        inputs["m1"], inputs["v1"], inputs["w2"], inputs["b2"], inputs["g2"],
        inputs["be2"], inputs["m2"], inputs["v2"],
    )
    nc = _get_nc()
    res = run_bass_kernel_spmd(
        nc, in_maps, core_ids=list(range(NCORES)), trace=trace, **kwargs
    )
    return res


def kernel(x, length, w1, b1, g1, be1, m1, v1, w2, b2, g2, be2, m2, v2):
    inputs = dict(
        x=x, length=length, w1=w1, b1=b1, g1=g1, be1=be1, m1=m1, v1=v1,
        w2=w2, b2=b2, g2=g2, be2=be2, m2=m2, v2=v2,
    )
    res = run_on_device(inputs, trace=False)
    return _postprocess(res.results, length)
